# revision 34
# baseline (speedup 1.0000x reference)
"""AttentionPairBias Trainium2 Bass kernel (v2).

Problem: nn_AttentionPairBias_49486613184627
  B=2, N=1024, D=768, E=128, H=16, HD=48.

Sharding: query-row (i) sharding across 8 cores. Core c handles rows
i in [c*128, (c+1)*128) for both batches; reads its edge_embed shard
(fp16) plus full k_in, and produces its (2,128,768) output slice.

Design notes (final):
  - edge shipped host-transposed (B, IS, E, N) in fp8-e4m3: [e, j]
    tiles load as plain DMAs with 1KB-contiguous partition lines.
  - bias matmul: lhsT = es tile (weights, fp8), rhs = wza 17 cols =
    16*(ln_g*Wz - c1/128) | 16.0. Mean-centering is folded into the
    weights (sum_e x*(w - c1/128) == P - mu*c1 exactly); the x16
    prescale keeps fp8 weight resolution and is folded back via
    rstd/16 (Exp bias = -ln 16). The LayerNorm beta term is constant
    along j => softmax-invariant => dropped. Col 16 gives sum_e x.
  - sum_e x^2: es^2 computed 6/4/6 on ACT/DVE/Pool (all three share
    the square; ACT's table set natural_log_exp_and_others holds
    Identity/Exp/Ln/Square so there is no table thrash), then 1-col
    ones matmuls accumulate into one psum bank per half-sweep; stats
    read it straight from psum. var = s2/128 - (s1/2048)^2, rstd via
    ACT Ln/Exp. bias = rstd * P' (single broadcast multiply per half).
  - k/v projections are j-sharded across the 8 cores and AllGathered
    through DRAM (collective emitted during the b=0 sweep; readback
    DMAs spread across the second half-sweep to avoid starving the
    edge-tile stream).
  - softmax sum folded into the o-matmul: v heads are stored 49 wide
    with col 48 = 1.0, so opsum col 48 = sum_j exp. No PE
    ones-reductions, no tiny transposes; go^T via XBAR DMA transpose.
  - v/g/o/Wo paths unpadded (768); q/k stay HDP=64-padded for the
    64-partition score matmul slices.
"""

import os
import sys

import ml_dtypes
import numpy as np

for _p in ("/opt/trn_rl_repo",):
    if _p not in sys.path:
        sys.path.insert(0, _p)

import concourse.bacc as bacc
import concourse.bass as bass
import concourse.mybir as mybir
import concourse.tile as tile
from concourse.bass_utils import run_bass_kernel_spmd

F16 = mybir.dt.float16
F32 = mybir.dt.float32
F8 = mybir.dt.float8e4
AF = mybir.ActivationFunctionType
ALU = mybir.AluOpType

B, N, D, E, H = 2, 1024, 768, 128, 16
HD = 48
HDP = 64              # padded head dim (q/k only)
DP = H * HDP          # 1024 padded model dim (q/k only)
VW = HD + 1           # 49: v head width with ones column
NC = 8                # cores
IS = N // NC          # 128 i-rows per core per batch
JC = N // 128         # 8 j-chunks
MC = D // 128         # 6 contraction chunks of 128 over D
IB = 8                # i-batch for stats/fixup
EPS = 1e-5

_BUILT = None
LAST_RESULTS = None   # BassKernelResults of last run (for test.py)


def _build_program():
    nc = bacc.Bacc(
        "TRN2",
        target_bir_lowering=False,
        debug=False,
        enable_asserts=False,
        num_devices=NC,
    )

    # ---------------- DRAM I/O ----------------
    d_edge = nc.dram_tensor("e", (B, IS, E, N), F8, kind="ExternalInput").ap()
    d_xt = nc.dram_tensor("xt", (B, D, IS), F16, kind="ExternalInput").ap()
    d_kin = nc.dram_tensor("kin", (B, D, IS), F16, kind="ExternalInput").ap()
    KSH = B * 8 * 128          # 2048: k shard cols (b, m, j_own)
    VSH = B * H * VW           # 1568: v shard cols (b, h, w)
    d_sh = nc.dram_tensor("kvsh", (128, KSH + VSH), F16, kind="Internal").ap()
    d_g = nc.dram_tensor(
        "kvg", (NC, 128, KSH + VSH), F16, kind="Internal", addr_space="Shared"
    ).ap()
    d_wq = nc.dram_tensor("wq", (D, DP), F16, kind="ExternalInput").ap()
    d_wk = nc.dram_tensor("wk", (D, DP), F16, kind="ExternalInput").ap()
    d_wv = nc.dram_tensor("wv", (D, D), F16, kind="ExternalInput").ap()
    d_wg = nc.dram_tensor("wg", (D, D), F16, kind="ExternalInput").ap()
    d_wo = nc.dram_tensor("wo", (D, D), F16, kind="ExternalInput").ap()
    d_bq = nc.dram_tensor("bq", (DP // 128, 128), F32, kind="ExternalInput").ap()
    d_wza = nc.dram_tensor("wza", (E, 17), F8, kind="ExternalInput").ap()
    d_out = nc.dram_tensor("o", (B, IS, D), F32, kind="ExternalOutput").ap()

    from contextlib import ExitStack

    with tile.TileContext(nc) as tc, ExitStack() as es:
        def pool(**kw):
            return es.enter_context(tc.tile_pool(**kw))

        # ---- persistent SBUF (whole kernel) ----
        constp = pool(name="const", bufs=1)
        ktpp = pool(name="ktp", bufs=1)
        vallp = pool(name="vall", bufs=1)
        qtpp = pool(name="qtp", bufs=1)
        gallp = pool(name="gall", bufs=1)
        wosbp = pool(name="wo_sb", bufs=1)
        # main-phase sweep pools allocated up front so the bias sweep
        # overlaps phase 0 instead of waiting on the pool-close barrier
        abufp = pool(name="abuf", bufs=2)
        esp = pool(name="es", bufs=8)
        es2p = pool(name="es2", bufs=6)
        statsp = pool(name="stats", bufs=2)
        smallp = pool(name="small", bufs=2)
        # phase-0-only pools in their own stack, closed after phase 0
        es0 = es.enter_context(ExitStack())
        wchp = es0.enter_context(tc.tile_pool(name="wchunk", bufs=6))
        kinchp = es0.enter_context(tc.tile_pool(name="kinchunk", bufs=12))
        kvshp = es0.enter_context(tc.tile_pool(name="kvshp", bufs=1))
        gwork = es0.enter_context(tc.tile_pool(name="gwork", bufs=1))
        # ---- PSUM pools ----
        mmps = pool(name="mm_ps", bufs=2, space="PSUM")   # [128,<=512] f32
        ppps = pool(name="pp_ps", bufs=4, space="PSUM")   # [128,144] f32
        ops = pool(name="o_ps", bufs=1, space="PSUM")     # [128,392] f32
        s2pp = pool(name="s2_ps", bufs=1, space="PSUM")   # [128,512] f32

        # ============ constants ============
        wza = constp.tile([E, 17], F8)
        nc.sync.dma_start(wza[:], d_wza[:, :])
        bqp = constp.tile([128, DP // 128], F32)
        nc.sync.dma_start(bqp[:], d_bq.rearrange("m p -> p m"))
        onesc = constp.tile([128, 1], F16)
        nc.vector.memset(onesc[:], 1.0)
        epsc = constp.tile([128, 1], F32)
        nc.vector.memset(epsc[:], EPS)
        # -ln(16): folds away the x16 host prescale of wza via exp()
        nl16c = constp.tile([128, 1], F32)
        nc.vector.memset(nl16c[:], -2.772588722239781)

        # persistent activation buffers
        # ktp: [b][m] 128 x 1024 (d' rows, j cols), fp16
        ktp = ktpp.tile([128, B * 8 * 1024], F16)
        ktp3 = ktp[:].rearrange("p (b m j) -> p b m j", b=B, m=8)
        # v: [b][jt] 128 x (16*49) (j rows, head-packed cols+ones), fp16
        vall = vallp.tile([128, B * 8 * H * VW], F16)
        vall4 = vall[:].rearrange(
            "p (b jt h w) -> p b jt h w", b=B, jt=8, h=H
        )
        # qtp: [m] 128 x (b,i), fp16
        qtp = qtpp.tile([128, 8 * B * IS], F16)
        qtp3 = qtp[:].rearrange("p (m b i) -> p m b i", m=8, b=B)
        # g: [b] 128(i) x 768, fp16
        gall = gallp.tile([128, B * D], F16)
        gall2 = gall[:].rearrange("p (b d) -> p b d", b=B)
        # wo chunks: [cc] 128 x 768 fp16
        wosb = wosbp.tile([128, MC * D], F16)
        wosb2 = wosb[:].rearrange("p (c d) -> p c d", c=MC)
        nc.sync.dma_start(wosb2, d_wo.rearrange("(c p) d -> p c d", p=128))
        # xt tiles: [c] 128(d-row) x (b,i)
        xts = constp.tile([128, MC * B * IS], F16)
        xts3 = xts[:].rearrange("p (c b i) -> p c b i", c=MC, b=B)
        for b in range(B):
            for c in range(MC):
                nc.sync.dma_start(
                    xts3[:, c, b, :], d_xt[b, c * 128:(c + 1) * 128, :]
                )

        # ============ phase 0: projections ============
        def load_chunks(dram, tag, width):
            ts = []
            for c in range(MC):
                t = wchp.tile([128, width], F16, tag=tag)
                nc.sync.dma_start(t[:], dram[c * 128:(c + 1) * 128, :])
                ts.append(t)
            return ts

        kin_sb = {}
        for b in range(B):
            kin_sb[b] = []
            for c in range(MC):
                t = kinchp.tile([128, IS], F16, tag="kin")
                nc.sync.dma_start(t[:], d_kin[b, c * 128:(c + 1) * 128, :])
                kin_sb[b].append(t)

        # q projection (both b at once; xts free dim is (b,i))
        wq_sb = load_chunks(d_wq, "w", DP)
        for m in range(8):
            qps = mmps.tile([128, B * IS], F32, tag="sc")
            for c in range(MC):
                nc.tensor.matmul(
                    qps[:],
                    wq_sb[c][:, m * 128:(m + 1) * 128],
                    xts3[:, c, :, :],
                    start=(c == 0),
                    stop=(c == MC - 1),
                )
            nc.scalar.activation(
                qtp3[:, m, :, :], qps[:],
                AF.Identity, bias=bqp[:, m:m + 1], scale=1.0,
            )

        # k/v: each core projects only its own 128 j-rows, then the
        # shards are AllGathered through DRAM (j-sharded tensor parallel).
        kvsh = kvshp.tile([128, KSH + VSH], F16, tag="kvsh")
        kvk = kvsh[:, 0:KSH].rearrange("p (b m j) -> p b m j", b=B, m=8)
        kvv = kvsh[:, KSH:].rearrange("p (b h w) -> p b h w", b=B, h=H)
        wk_sb = load_chunks(d_wk, "w", DP)
        for b in range(B):
            for m in range(8):
                kps = mmps.tile([128, IS], F32, tag="sc")
                for c in range(MC):
                    nc.tensor.matmul(
                        kps[:],
                        wk_sb[c][:, m * 128:(m + 1) * 128],
                        kin_sb[b][c][:],
                        start=(c == 0),
                        stop=(c == MC - 1),
                    )
                nc.scalar.activation(
                    kvk[:, b, m, :], kps[:],
                    AF.Identity, bias=0.0, scale=1.0,
                )

        wv_sb = load_chunks(d_wv, "w", D)
        for b in range(B):
            for nb in range(2):  # halves of 384 = 8 heads each
                vps = mmps.tile([128, 384], F32, tag="sc")
                for c in range(MC):
                    nc.tensor.matmul(
                        vps[:],
                        kin_sb[b][c][:],
                        wv_sb[c][:, nb * 384:(nb + 1) * 384],
                        start=(c == 0),
                        stop=(c == MC - 1),
                    )
                nc.scalar.activation(
                    kvv[:, b, nb * 8:(nb + 1) * 8, 0:HD],
                    vps[:].rearrange("p (h d) -> p h d", h=8),
                    AF.Identity, bias=0.0, scale=1.0,
                )
            nc.vector.memset(kvv[:, b, :, HD:VW], 1.0)

        # shard -> DRAM now; AllGather + readback deferred into the
        # bias sweep so the DMA queues aren't head-of-line blocked on the
        # collective semaphore before the edge loads can start.
        nc.sync.dma_start(d_sh[:, :], kvsh[:])

        def emit_kv_collective():
            nc.gpsimd.collective_compute(
                "AllGather", ALU.bypass,
                replica_groups=[list(range(NC))],
                ins=[d_sh[:, :]], outs=[d_g[:, :, :]],
            )

        def kv_gather_dmas():
            for b in range(B):
                kb = b * 8 * 128
                vb = KSH + b * H * VW
                for c in range(NC):
                    yield lambda b=b, c=c, kb=kb: nc.sync.dma_start(
                        ktp3[:, b, :, c * 128:(c + 1) * 128],
                        d_g[c, :, kb:kb + 8 * 128].rearrange(
                            "p (m j) -> p m j", m=8
                        ),
                    )
                    yield lambda b=b, c=c, vb=vb: nc.sync.dma_start(
                        vall4[:, b, c, :, :],
                        d_g[c, :, vb:vb + H * VW].rearrange(
                            "p (h w) -> p h w", h=H
                        ),
                    )

        kv_dma_iter = kv_gather_dmas()

        # g = 1/(1+exp(-z)); wg is pre-negated on host -> psum = -z
        wg_sb = load_chunks(d_wg, "w", D)
        for b in range(B):
            gtmp = gwork.tile([128, D], F32, tag="gtmp")
            for nb in range(2):
                gps = mmps.tile([128, 384], F32, tag="sc")
                for c in range(MC):
                    nc.tensor.matmul(
                        gps[:],
                        xts3[:, c, b, :],
                        wg_sb[c][:, nb * 384:(nb + 1) * 384],
                        start=(c == 0),
                        stop=(c == MC - 1),
                    )
                nc.scalar.activation(
                    gtmp[:, nb * 384:(nb + 1) * 384], gps[:],
                    AF.Exp, bias=0.0, scale=1.0,
                )
            nc.vector.tensor_scalar_add(gtmp[:], gtmp[:], 1.0)
            grec = gwork.tile([128, D], F32, tag="grec")
            nc.vector.reciprocal(grec[:], gtmp[:])
            nc.vector.tensor_copy(gall2[:, b, :], grec[:])

        # ---- close phase-0 pools, open main-phase pools ----
        es0.close()
        expsbp = pool(name="expsb", bufs=2)
        oasmp = pool(name="oasm", bufs=2)
        outsbp = pool(name="outsb", bufs=2)

        IH = IS // 2          # 64: i-half for stats batching

        # ============ main: per-b bias + attention ============
        for b in range(B):
            # bias addend buffer: [p=j][jc][h][i] fp16, per b
            abuf = abufp.tile([128, IS * JC * H], F16, tag="ab")
            abuf4 = abuf[:].rearrange("p (i jc h) -> p i jc h", i=IS, jc=JC)
            # ---- bias sweep over i, stats batched per half ----
            for half_i in range(2):
                s1b = statsp.tile([128, IH * JC], F32, tag="s1")
                s1_3 = s1b[:].rearrange("p (i jc) -> p i jc", i=IH)
                s2h = s2pp.tile([128, IH * JC], F32, tag="s2p")
                s2h3 = s2h[:].rearrange("p (i jc) -> p i jc", i=IH)
                for ii in range(0, IH, 2):
                    i = half_i * IH + ii
                    if b == 0 and half_i == 0 and ii == 24:
                        emit_kv_collective()
                    # two i's share one psum tile and one extract pass;
                    # sum-x^2 goes to its own psum so the P' extract does
                    # not wait on the square side-chain
                    pp = ppps.tile([128, 2 * JC * 17], F32, tag="pp")
                    pp4 = pp[:].rearrange(
                        "p (u jc s) -> p u jc s", u=2, jc=JC
                    )

                    for u in range(2):
                        # [e, j] tile: host-transposed edge, 1KB lines
                        est = esp.tile([128, N], F8, tag="es")
                        nc.sync.dma_start(est[:], d_edge[b, i + u, :, :])
                        es3 = est[:].rearrange("p (jc j) -> p jc j", jc=JC)
                        es2t = es2p.tile([128, N], F16, tag="es2")
                        # square split 6/4/6 across ACT / DVE / Pool;
                        # skip Pool in the window where the collective
                        # blocks its queue
                        iu = i + u
                        r = iu % 16
                        if b == 0 and 48 <= iu < 96:
                            sq = nc.scalar if iu % 3 != 2 else None
                        elif r < 6:
                            sq = nc.scalar
                        elif r < 10:
                            sq = None
                        else:
                            sq = nc.gpsimd
                        if sq is nc.scalar:
                            nc.scalar.activation(
                                es2t[:], est[:], AF.Square,
                                bias=0.0, scale=1.0,
                            )
                        elif sq is None:
                            nc.vector.tensor_tensor(
                                es2t[:], est[:], est[:], ALU.mult
                            )
                        else:
                            nc.gpsimd.tensor_tensor(
                                es2t[:], est[:], est[:], ALU.mult
                            )
                        es23 = es2t[:].rearrange("p (jc j) -> p jc j", jc=JC)
                        for jc in range(JC):
                            # P'[j,0:16] bias (mean-centered), 16 = sum x
                            nc.tensor.matmul(
                                pp4[:, u, jc, :], es3[:, jc, :], wza[:],
                                start=True, stop=True,
                            )
                            # sum x^2 (side-chain psum, one bank/half)
                            nc.tensor.matmul(
                                s2h3[:, ii + u, jc:jc + 1], es23[:, jc, :],
                                onesc[:],
                                start=True, stop=True,
                            )
                    # extract pair: bias cols -> abuf (ACT), stats (DVE)
                    nc.scalar.activation(
                        abuf4[:, i:i + 2, :, :], pp4[:, :, :, 0:16],
                        AF.Identity, bias=0.0, scale=1.0,
                    )
                    nc.vector.tensor_copy(
                        s1_3[:, ii:ii + 2, :], pp4[:, :, :, 16]
                    )
                    if b == 0 and half_i == 1:
                        for fn in (next(kv_dma_iter, None),
                                   next(kv_dma_iter, None)):
                            if fn is not None:
                                fn()

                # ---- batched stats: var = s2/128 - (s1/128)^2 ----
                mu = smallp.tile([128, IH * JC], F32, tag="mu")
                nc.vector.tensor_scalar_mul(
                    mu[:], s1b[:], 1.0 / (128.0 * 16.0)
                )
                var = smallp.tile([128, IH * JC], F32, tag="var")
                nc.vector.tensor_scalar_mul(var[:], s2h[:], 1.0 / 128.0)
                mu2 = smallp.tile([128, IH * JC], F32, tag="mu2")
                nc.vector.tensor_tensor(mu2[:], mu[:], mu[:], ALU.mult)
                nc.vector.tensor_tensor(var[:], var[:], mu2[:], ALU.subtract)
                rstd = smallp.tile([128, IH * JC], F32, tag="rstd")
                nc.scalar.activation(
                    rstd[:], var[:], AF.Ln, bias=epsc[:, :], scale=1.0
                )
                # rstd/16 folds away the x16 host prescale of wza
                nc.scalar.activation(
                    rstd[:], rstd[:], AF.Exp, bias=nl16c[:, :], scale=-0.5,
                )
                rstd3 = rstd[:].rearrange("p (i jc) -> p i jc", i=IH)
                # ---- fixup: abuf *= rstd (in place, one op per half) ----
                ab_blk = abuf4[:, half_i * IH:(half_i + 1) * IH, :, :]
                r_bc = rstd3.rearrange(
                    "p i jc -> p i jc ()"
                ).broadcast_to((128, IH, JC, H))
                nc.vector.tensor_tensor(ab_blk, ab_blk, r_bc, ALU.mult)

            # ---- attention for this b ----
            oasm = oasmp.tile([128, D], F16, tag="oa")
            for hg in range(2):
                opsum = ops.tile([128, 8 * VW], F32, tag="ops")
                for hh in range(8):
                    h = hg * 8 + hh
                    m = h // 2
                    prow = (h % 2) * 64
                    expsb = expsbp.tile([128, N], F16, tag="ex")
                    ex3 = expsb[:].rearrange("p (jc i) -> p jc i", jc=JC)
                    for half in range(2):
                        scp = mmps.tile([128, 512], F32, tag="sc")
                        sc3 = scp[:].rearrange("p (jc i) -> p jc i", jc=4)
                        for sj in range(4):
                            jc = half * 4 + sj
                            nc.tensor.matmul(
                                sc3[:, sj, :],
                                ktp3[:, b, m, jc * 128:(jc + 1) * 128][
                                    prow:prow + 64, :
                                ],
                                qtp3[:, m, b, :][prow:prow + 64, :],
                                start=True, stop=True,
                            )
                        # add pair bias (DVE, psum rmw)
                        nc.vector.tensor_tensor(
                            sc3[:, :, :], sc3[:, :, :],
                            abuf4[:, :, half * 4:(half + 1) * 4, h].rearrange(
                                "p i jc -> p jc i"
                            ),
                            ALU.add,
                        )
                        # exp -> sbuf fp16
                        nc.scalar.activation(
                            ex3[:, half * 4:(half + 1) * 4, :], sc3,
                            AF.Exp, bias=0.0, scale=1.0,
                        )
                    # o = exp^T @ v (accumulate over jc); col 48 = sum exp
                    for jc in range(JC):
                        nc.tensor.matmul(
                            opsum[:, hh * VW:(hh + 1) * VW],
                            ex3[:, jc, :],
                            vall4[:, b, jc, h, :],
                            start=(jc == 0), stop=(jc == JC - 1),
                        )
                # 1/s for the group from opsum col-48 stripes
                sinv = smallp.tile([128, 8], F32, tag="sinv")
                nc.vector.reciprocal(
                    sinv[:],
                    opsum[:].rearrange("p (h w) -> p h w", h=8)[:, :, HD],
                )
                for hh in range(8):
                    h = hg * 8 + hh
                    nc.scalar.activation(
                        oasm[:, h * HD:(h + 1) * HD],
                        opsum[:, hh * VW:hh * VW + HD],
                        AF.Identity, bias=0.0, scale=sinv[:, hh:hh + 1],
                    )
            # go = g * o  (fp16)
            go = oasmp.tile([128, D], F16, tag="go")
            nc.vector.tensor_tensor(go[:], oasm[:], gall2[:, b, :], ALU.mult)
            # transpose go -> goT chunks [d rows, i cols] via XBAR DMA
            goT = oasmp.tile([128, D], F16, tag="goT")
            go3 = go[:].rearrange("p (c q) -> p c q", c=MC)
            goT3 = goT[:].rearrange("p (c q) -> p c q", c=MC)
            for cc in range(MC):
                nc.sync.dma_start_transpose(goT3[:, cc, :], go3[:, cc, :])
            # final: out[i, :768] = goT.T @ wo
            outsb = outsbp.tile([128, D], F32, tag="ou")
            for nb, nsz in ((0, 512), (1, 256)):
                fps = mmps.tile([128, 512], F32, tag="sc")
                for cc in range(MC):
                    nc.tensor.matmul(
                        fps[:, 0:nsz],
                        goT3[:, cc, :],
                        wosb2[:, cc, nb * 512:nb * 512 + nsz],
                        start=(cc == 0), stop=(cc == MC - 1),
                    )
                nc.scalar.activation(
                    outsb[:, nb * 512:nb * 512 + nsz], fps[:, 0:nsz],
                    AF.Identity, bias=0.0, scale=1.0,
                )
            nc.sync.dma_start(d_out[b, :, :], outsb[:])

    nc.compile()
    return nc


def _prep_host(inputs):
    """Build per-core input maps (host-side layout marshalling only)."""
    node = np.asarray(inputs["node_embed"], np.float32)
    edge = np.asarray(inputs["edge_embed"], np.float32)
    mask = np.asarray(inputs["node_mask"])
    k_in = np.asarray(inputs["k_in"], np.float32)
    Wq = np.asarray(inputs["Wq"], np.float32)
    bq = np.asarray(inputs["bq"], np.float32)
    Wk = np.asarray(inputs["Wk"], np.float32)
    Wv = np.asarray(inputs["Wv"], np.float32)
    Wg = np.asarray(inputs["Wg"], np.float32)
    ln_g = np.asarray(inputs["ln_g"], np.float32)
    ln_b = np.asarray(inputs["ln_b"], np.float32)
    Wz = np.asarray(inputs["Wz"], np.float32)
    Wo = np.asarray(inputs["Wo"], np.float32)

    assert np.all(np.asarray(mask) == 1), "mask path not implemented"

    scale = 1.0 / np.sqrt(HD)

    def padhead_rows(W):  # (768,768) -> (1024,768): out' rows padded
        Wp = np.zeros((DP, D), np.float32)
        for h in range(H):
            Wp[h * HDP:h * HDP + HD] = W[h * HD:(h + 1) * HD]
        return Wp

    wqT = (padhead_rows(Wq) * scale).T.astype(np.float16).copy()
    wkT = padhead_rows(Wk).T.astype(np.float16).copy()
    wvT = Wv.T.astype(np.float16).copy()       # (768, 768) head-packed
    wgT = (-Wg).T.astype(np.float16).copy()    # negated for sigmoid
    woT = Wo.T.astype(np.float16).copy()       # (768 go-dim, 768 out)

    bqp = np.zeros((DP,), np.float32)
    for h in range(H):
        bqp[h * HDP:h * HDP + HD] = bq[h * HD:(h + 1) * HD] * scale
    bqp = bqp.reshape(DP // 128, 128)

    # bias weights with mean-centering fold; col 16 = ones (sum x)
    w = ln_g[:, None] * Wz                      # (E, 16)
    c1 = w.sum(axis=0)                          # (16,)
    wza = np.zeros((E, 17), np.float32)
    # x16 prescale keeps the fp8 weights in e4m3's resolution sweet
    # spot; folded back via rstd/16 (Exp bias) on device.
    wza[:, :16] = (w - c1[None, :] / 128.0) * 16.0
    wza[:, 16] = 16.0
    wza16 = wza.astype(ml_dtypes.float8_e4m3fn)
    # (ln_b @ Wz is constant along j -> softmax-invariant -> dropped)

    xt = node.transpose(0, 2, 1).astype(np.float16).copy()     # (B, D, N)
    kinT = k_in.transpose(0, 2, 1).astype(np.float16).copy()   # (B, D, N)
    # edge host-transposed to (B, N_i, E, N_j) fp8 (lazy view; the
    # per-core ascontiguousarray below materializes each shard)
    edge16 = edge.astype(ml_dtypes.float8_e4m3fn).transpose(0, 1, 3, 2)

    in_maps = []
    for c in range(NC):
        i0 = c * IS
        in_maps.append({
            "e": np.ascontiguousarray(edge16[:, i0:i0 + IS]),
            "xt": np.ascontiguousarray(xt[:, :, i0:i0 + IS]),
            "kin": np.ascontiguousarray(kinT[:, :, i0:i0 + IS]),
            "wq": wqT, "wk": wkT, "wv": wvT, "wg": wgT, "wo": woT,
            "bq": bqp, "wza": wza16,
        })
    return in_maps


def kernel(**inputs):
    global _BUILT, LAST_RESULTS
    if _BUILT is None:
        _BUILT = _build_program()
    nc = _BUILT
    in_maps = _prep_host(inputs)
    res = run_bass_kernel_spmd(
        nc, in_maps, core_ids=list(range(NC)),
        trace=bool(int(os.environ.get("KERNEL_TRACE", "0"))),
    )
    LAST_RESULTS = res
    out = np.empty((B, N, D), np.float32)
    for c in range(NC):
        out[:, c * IS:(c + 1) * IS] = res.results[c]["o"]
    return out


if __name__ == "__main__":
    sys.path.insert(0, os.path.dirname(os.path.abspath(__file__)))
    import reference
    inputs = {k: np.asarray(v) for k, v in reference.setup_inputs().items()}
    got = kernel(**inputs)
    want = np.asarray(reference.reference(**reference.setup_inputs()))
    err = np.abs(got - want)
    rel = err.max() / np.abs(want).max()
    print("max abs err:", err.max(), "rel:", rel)
